# revision 32
# baseline (speedup 1.0000x reference)
"""Trainium2 Bass kernel for batch-all triplet margin loss (N=512, D=128).

Math:
  dist[i,g] = ||x_i - x_g||  (the reference's +1e-6 inside the norm shifts
  d2 by ~3e-5 -- far below bf16 noise, so it is dropped)
  loss = mean over valid (i,j,g) of relu(dist[i,j] - dist[i,g] + margin)
  valid: labels[j]==labels[i], j != i, labels[g] != labels[i]

Device strategy (SPMD over 8 cores, 64 anchors each, rows duplicated x2 so
each main-loop pass covers two positive ordinals):
  - all matmul operands are bf16 (1 cycle/row on PE, half the DMA bytes).
    X is rounded to bf16 once on the host and r = ||x||^2 is computed FROM
    the rounded values, so d2(i,i) lands exactly on the +0.25 fudge.
  - d2 blocks are built in PSUM from two matmuls: the -2 X_a X^T product,
    plus a low-rank "augment" matmul whose rows carry r_g (hi+lo bf16
    split) and a 2^120 same-class mask (rank-16: one row per class).
    r_i enters exactly via the per-partition fp32 bias of the sqrt
    activation.  Masked columns become ~1.15e18 after sqrt and never pass
    the relu.
  - transposed-path d2t (for positive distances) gets r_g via per-chunk
    augment rows, r_i(+0.25) via hi/lo rhs rows; +0.25 on ALL entries so
    d2t(i,i) can't round negative (shifts every a by +0.008 -- well under
    tolerance).
  - positive distances are gathered with 0/1 rank-selector matmuls (even/
    odd ordinal) into a packed [128, umax] bias tile, +margin.
  - main loop: one instruction per ordinal-pair u:
      DVE: tensor_scalar out=(Bneg - a_u) min 0, accum_out = col sum
           (bf16 SBUF operands -> 4x_2p DVE mode, ~165ns/pass)
      ACT (a few passes): relu(-Bneg + a_u) with accum_out
  - accumulator columns DMA out once; host reduces (DVE cols are <=0,
    ACT cols >=0, so |col sum| is sign-robust), divides by the host-
    computed triplet count.
"""

import numpy as np
import ml_dtypes

BF = ml_dtypes.bfloat16
N, D, C = 512, 128, 16
NCORES = 8
APC = N // NCORES  # 64 anchors per core
# same-class mask added to d2.  2^20 is chosen so masked distances come out
# EXACTLY 1024 in bf16 (sqrt(2^20 + d2) = 1024*(1 + d2/2^21) rounds to 1024
# for d2 <= ~2000, bf16 step at 1024 is 8): the positive-distance gather can
# then run on the UNMASKED-by-PSELT dpost and the known 1024*c garbage is
# subtracted host-side via a per-(anchor,u) correction tile.
MASK = float(2.0**20)
MVAL = 1024.0  # exact bf16 value of every masked dpost entry
FUDGE = 0.25  # keeps d2t(i,i) positive against bf16/psum rounding;
# shifts every positive distance by +FUDGE/(2d) ~ +0.008

_CACHE = {}


def _build_program(umax, margin, act_us):
    import concourse.bacc as bacc
    import concourse.tile as tile
    from concourse import mybir

    fp32 = mybir.dt.float32
    bf16 = mybir.dt.bfloat16
    AF = mybir.ActivationFunctionType
    OP = mybir.AluOpType

    c_ge = 0
    c_go = 4 * umax
    selw = 8 * umax

    nc = bacc.Bacc("TRN2", target_bir_lowering=False, debug=False)
    pa = nc.declare_dram_parameter("pa", [128, 640], bf16, isOutput=False)
    pk2 = nc.declare_dram_parameter("pk2", [21, 1280], bf16, isOutput=False)
    sel = nc.declare_dram_parameter("sel", [128, selw], bf16, isOutput=False)
    pf = nc.declare_dram_parameter("pf", [128, 1 + umax], fp32, isOutput=False)
    acc_out = nc.declare_dram_parameter(
        "acc", [128, umax + 1], fp32, isOutput=True
    )

    with tile.TileContext(nc) as tc:
        with (
            tc.tile_pool(name="io", bufs=1) as io,
            tc.tile_pool(name="work", bufs=1) as work,
            tc.tile_pool(name="psum", bufs=1, space="PSUM") as psum,
        ):
            t_pa = io.tile([128, 640], bf16)
            t_pk2 = io.tile([21, 1280], bf16)
            t_sel = io.tile([128, selw], bf16)
            t_pf = io.tile([128, 1 + umax], fp32)
            # keep the ACT queue free of DMA issues: its sequencer must get
            # to the act-table loads as early as possible.  pa goes on the
            # gpsimd SWDGE ring whose descriptor prep starts before the
            # HWDGE rings get through their fixed overheads.
            nc.sync.dma_start(t_pa[:], pa[:])
            nc.scalar.dma_start(t_pk2[:], pk2[:])
            nc.sync.dma_start(t_sel[:], sel[:])
            nc.scalar.dma_start(t_pf[:], pf[:])

            anchors = t_pa[:, 0:APC]       # X_I^T [d, 64]
            xia = t_pa[:, 0:128]           # X_Idup^T [d, 128]
            xga = t_pa[:, 128:640]         # -2 X^T [d, 512]

            # ---- transposed d2t: [g(part, 4 chunks), anchor(64)] ----
            # product and augment per chunk must stay adjacent: the psum
            # bank allows only ONE pending accumulation group at a time
            p_d2t = psum.tile([128, 4 * APC], fp32, tag="d2t")
            for q in range(4):
                nc.tensor.matmul(
                    p_d2t[:, q * APC : (q + 1) * APC],
                    t_pa[:, 128 + q * 128 : 128 + (q + 1) * 128],
                    anchors,
                    start=True,
                    stop=False,
                )
                nc.tensor.matmul(
                    p_d2t[:, q * APC : (q + 1) * APC],
                    t_pk2[0:21, 640 + q * 128 : 640 + (q + 1) * 128],
                    t_pk2[0:21, 1152:1216],
                    start=False,
                    stop=True,
                )
            # ---- big d2 block for anchors dup x2: [slot(128), g(512)] ----
            p_d2 = psum.tile([128, N], fp32, tag="d2")
            nc.tensor.matmul(p_d2[:], xia, xga, start=True, stop=False)
            nc.tensor.matmul(
                p_d2[:], t_pk2[0:18, 0:128], t_pk2[0:18, 128:640],
                start=False, stop=True,
            )

            # ---- positive distances: sqrt, mask to same-class, gather ----
            t_dpost = work.tile([128, 4 * APC], bf16, tag="dpost")
            nc.scalar.activation(t_dpost[:], p_d2t[:], AF.Sqrt)
            t_bneg = work.tile([128, N], bf16, tag="bneg")
            nc.scalar.activation(t_bneg[:], p_d2[:], AF.Sqrt, bias=t_pf[:, 0:1])

            p_ab = psum.tile([128, umax], fp32, tag="ab")
            for q in range(4):
                nc.tensor.matmul(
                    p_ab[:APC, :],
                    t_dpost[:, q * APC : (q + 1) * APC],
                    t_sel[:, c_ge + q * umax : c_ge + (q + 1) * umax],
                    start=(q == 0),
                    stop=(q == 3),
                )
            for q in range(4):
                nc.tensor.matmul(
                    p_ab[APC:, :],
                    t_dpost[:, q * APC : (q + 1) * APC],
                    t_sel[:, c_go + q * umax : c_go + (q + 1) * umax],
                    start=(q == 0),
                    stop=(q == 3),
                )
            t_ab = work.tile([128, umax], fp32, tag="ab2")
            nc.vector.tensor_tensor(
                t_ab[:], p_ab[:], t_pf[:, 1 : 1 + umax], op=OP.subtract
            )

            # ---- main loop ----
            # DVE accum_out reduces with op1, so a sum-accumulate needs
            # op0=max/op1=add: acc_u = sum_g max(d_g, a_u).  The host gets
            # sum_g relu(a_u - d_g) = acc_u - rowsum(d) via one extra
            # rowsum column (ACT Identity with accum_out); masked columns
            # contribute exactly 1024 to both sides and cancel.
            # Ping-pong trash tiles per engine so consecutive passes have no
            # write-after-write dependency.
            t_acc = work.tile([128, umax + 1], fp32, tag="acc")
            t_trash_d = [
                work.tile([128, N], bf16, name=f"trd{i}", tag=f"trd{i}")
                for i in range(2)
            ]
            t_trash_a = [
                work.tile([128, N], bf16, name=f"tra{i}", tag=f"tra{i}")
                for i in range(2)
            ]
            # rowsum(d) column for the max-sum correction
            nc.scalar.activation(
                t_trash_a[1][:],
                t_bneg[:],
                AF.Identity,
                accum_out=t_acc[:, umax : umax + 1],
            )
            nd = na = 0
            for u in range(umax):
                if u in act_us:
                    nc.scalar.activation(
                        t_trash_a[na % 2][:],
                        t_bneg[:],
                        AF.Relu,
                        bias=t_ab[:, u : u + 1],
                        scale=-1.0,
                        accum_out=t_acc[:, u : u + 1],
                    )
                    na += 1
                else:
                    nc.vector.tensor_scalar(
                        t_trash_d[nd % 2][:],
                        t_bneg[:],
                        t_ab[:, u : u + 1],
                        None,
                        op0=OP.max,
                        op1=OP.add,
                        accum_out=t_acc[:, u : u + 1],
                    )
                    nd += 1

            nc.sync.dma_start(acc_out[:], t_acc[:])

    nc.finalize()
    return nc


def plan(outputs, labels, margin, n_act=4):
    """Build (nc, in_maps, umax, count); shared by kernel() and test."""
    X64 = np.asarray(outputs, dtype=np.float64)
    lab = np.asarray(labels).astype(np.int64).reshape(-1)
    margin = float(margin)
    assert X64.shape == (N, D) and lab.shape == (N,)

    Xb = X64.astype(BF)                      # round once
    Xw = Xb.astype(np.float64)               # exact value of the rounding
    r = (Xw * Xw).sum(1)                     # row norms of the rounded X

    m = np.bincount(lab, minlength=max(C, int(lab.max()) + 1))
    jmax = int(m.max())
    umax = (jmax + 1) // 2
    count = float(sum(int(mc) * (int(mc) - 1) * (N - int(mc)) for mc in m))

    rank = np.zeros(N, dtype=np.int64)
    cnt = {}
    for j in range(N):
        c = int(lab[j])
        rank[j] = cnt.get(c, 0)
        cnt[c] = cnt.get(c, 0) + 1
    G = np.zeros((N, 2 * umax), dtype=np.float64)
    G[np.arange(N), rank] = 1.0
    GE, GO = G[:, 0::2], G[:, 1::2]  # [512, umax] each

    n_act = max(0, min(n_act, umax))
    act_us = frozenset(range(umax - n_act, umax))
    global _LAST_ACT_US
    _LAST_ACT_US = act_us

    key = (umax, margin, act_us)
    if key not in _CACHE:
        _CACHE[key] = _build_program(umax, margin, act_us)
    nc = _CACHE[key]

    def chunked(A, cols):
        # [512, cols] -> [128, 4*cols] with chunk q at cols [q*cols:(q+1)*cols]
        return A.reshape(4, 128, cols).transpose(1, 0, 2).reshape(128, 4 * cols)

    # hi/lo bf16 split of r_g (rhs rows of the big augment matmul)
    r_hi = r.astype(BF)
    r_lo = (r - r_hi.astype(np.float64)).astype(BF)
    # transposed path: r_i + fudge, hi/lo
    v = r + FUDGE
    v_hi = v.astype(BF)
    v_lo = (v - v_hi.astype(np.float64)).astype(BF)

    onehot = (lab[None, :] == np.arange(C)[:, None])  # [16, 512]

    c_ge = 0
    c_go = 4 * umax
    selw = 8 * umax

    in_maps = []
    for c in range(NCORES):
        I = np.arange(c * APC, (c + 1) * APC)
        Idup = np.concatenate([I, I])

        PA = np.empty((128, 640), dtype=BF)
        PA[:, 0:128] = Xb[Idup].T
        PA[:, 128:640] = (-2.0 * Xw).astype(BF).T  # exact: power-of-2 scale

        PK2 = np.zeros((21, 1280), dtype=BF)
        # big augment: lhsT [18, 128] at cols 0:128, rhs [18, 512] at 128:640
        PK2[0, 0:128] = 1.0
        PK2[1, 0:128] = 1.0
        PK2[2:18, 0:128] = np.where(onehot[:, Idup], MASK, 0.0)
        PK2[0, 128:640] = r_hi
        PK2[1, 128:640] = r_lo
        PK2[2:18, 128:640] = onehot.astype(np.float64)
        # transposed augment: lhsT [21, 128] per chunk q at 640+q*128,
        # rhs [21, 64] at 1152:1216.  Rows 4..20 add MASK to CROSS-class
        # entries (M*1 - M*onehot(lab_g)*onehot(lab_a)) so dpost garbage is
        # exactly MVAL.
        for q in range(4):
            s = 640 + q * 128
            PK2[0, s : s + 128] = r_hi[q * 128 : (q + 1) * 128]
            PK2[1, s : s + 128] = r_lo[q * 128 : (q + 1) * 128]
            PK2[2, s : s + 128] = 1.0
            PK2[3, s : s + 128] = 1.0
            PK2[4, s : s + 128] = MASK
            PK2[5:21, s : s + 128] = np.where(
                onehot[:, q * 128 : (q + 1) * 128], -MASK, 0.0
            )
        PK2[0, 1152:1216] = 1.0
        PK2[1, 1152:1216] = 1.0
        PK2[2, 1152:1216] = v_hi[I]
        PK2[3, 1152:1216] = v_lo[I]
        PK2[4, 1152:1216] = 1.0
        PK2[5:21, 1152:1216] = onehot[:, I].astype(np.float64)

        SEL = np.empty((128, selw), dtype=BF)
        SEL[:, c_ge : c_ge + 4 * umax] = chunked(GE, umax)
        SEL[:, c_go : c_go + 4 * umax] = chunked(GO, umax)

        # correction tile: p_ab[p,u] = d_pos + MVAL*c where c counts OTHER
        # classes owning a rank-rho member; t_ab = p_ab - (MVAL*c - margin)
        PF = np.zeros((128, 1 + umax), dtype=np.float32)
        PF[:, 0] = r[Idup]
        rho = np.empty((128, umax), dtype=np.int64)
        uu = np.arange(umax)
        rho[:64] = 2 * uu[None, :]
        rho[64:] = 2 * uu[None, :] + 1
        nclasses_gt = np.zeros(N + 2, dtype=np.int64)  # nclasses_gt[rank] = #classes with m > rank
        for mc in m:
            nclasses_gt[: int(mc)] += 1
        own = m[lab[Idup]]  # own class size per slot
        cgar = nclasses_gt[rho] - (rho < own[:, None])
        PF[:, 1:] = (MVAL * cgar - margin).astype(np.float32)

        in_maps.append({"pa": PA, "pk2": PK2, "sel": SEL, "pf": PF})

    return nc, in_maps, umax, count


_LAST_ACT_US = frozenset()


def reduce_results(results, umax, count):
    # ACT columns hold relu sums directly; DVE columns hold
    # sum_g max(d_g, a_u) and need the rowsum column (index umax)
    # subtracted: sum_g relu(a_u - d_g) = acc_u - rowsum(d).
    total = 0.0
    for c in range(NCORES):
        acc = results[c]["acc"].astype(np.float64)  # [128, umax+1]
        rs = acc[:, umax].sum()
        for u in range(umax):
            cs = acc[:, u].sum()
            total += cs if u in _LAST_ACT_US else cs - rs
    return np.float32(total / count)


def kernel(outputs, labels, margin):
    from concourse.bass_utils import run_bass_kernel_spmd

    nc, in_maps, umax, count = plan(outputs, labels, margin)
    res = run_bass_kernel_spmd(nc, in_maps, list(range(NCORES)))
    loss = reduce_results(res.results, umax, count)
    return (loss, 0.0, 0.0, 0.0)


# revision 35
# speedup vs baseline: 1.0706x; 1.0706x over previous
"""Trainium2 Bass kernel for batch-all triplet margin loss (N=512, D=128).

Math:
  dist[i,g] = ||x_i - x_g||  (the reference's +1e-6 inside the norm shifts
  d2 by ~3e-5 -- far below bf16 noise, so it is dropped)
  loss = mean over valid (i,j,g) of relu(dist[i,j] - dist[i,g] + margin)
  valid: labels[j]==labels[i], j != i, labels[g] != labels[i]

Device strategy (SPMD over 8 cores, 64 anchors each, rows duplicated x2 so
each main-loop pass covers two positive ordinals):
  - all matmul operands are bf16 (1 cycle/row on PE, half the DMA bytes).
    X is rounded to bf16 once on the host and r = ||x||^2 is computed FROM
    the rounded values, so self-d2 lands exactly on the +FUDGE pad.
  - d2 blocks build in PSUM as (-2 X_a X^T product) + a low-rank augment
    matmul carrying r_g (hi/lo bf16 split) and a 2^20 same-class mask
    (rank-16, one row per class); r_i enters exactly via the per-partition
    fp32 bias of the sqrt activation.  Masked distances come out EXACTLY
    1024 in bf16.
  - transposed-path d2t (positive distances; partitions=g, cols=anchor)
    carries r_g via per-chunk augment lhsT rows, r_i+FUDGE via rhs rows,
    and the INVERTED (cross-class) mask, so rank-selector gathers of dpost
    pick d_pos plus exactly 1024*c garbage, c = #other classes owning a
    rank-rho member -- known on the host and subtracted via a correction
    tile (t_ab = p_ab - (1024*c - margin)).
  - main loop, one instruction per ordinal-pair u:
      DVE: tensor_scalar op0=max/op1=add: acc_u = sum_g max(d_g, a_u)
           (bf16 SBUF operands -> 4x_2p DVE mode, ~165ns/pass); the host
           recovers sum_g relu(a_u-d_g) = acc_u - rowsum(d) using a rowsum
           column (ACT Identity + accum_out); masked columns add exactly
           1024 to both sides and cancel.
      ACT: relu(-d + a_u) with accum_out (direct relu sums).
  - pass-count: the main tile needs only ceil(m2/2) passes (m2 = second
    largest class size).  The largest class's remaining ranks are handled
    by ONE tail pass per core: every core holds a replica of the big
    class's anchors (dup x2 <= 128 slots) with its own d2/dpost blocks,
    and core c's tail selector picks rank pair (2*umax_m + 2c, +1).
  - acc [128, umax_m + 2]: main cols | tail col (ACT relu) | rowsum col.
    One output DMA; host reduces and divides by the exact triplet count.
"""

import numpy as np
import ml_dtypes

BF = ml_dtypes.bfloat16
N, D, C = 512, 128, 16
NCORES = 8
APC = N // NCORES  # 64 anchors per core
# 2^20 -> masked distances are exactly bf16 1024 (sqrt(2^20+d2) rounds to
# 1024 for d2 <= ~2000 since the bf16 step at 1024 is 8)
MASK = float(2.0**20)
MVAL = 1024.0
FUDGE = 0.25  # keeps self-d2 positive against bf16/psum rounding
PADBIAS = 1.0e9  # tail-pad slots get bias -PADBIAS => relu contributes 0

_CACHE = {}


def _build_program(umax, margin, act_us):
    """umax here is umax_m (main ordinal-pair count); +2 acc columns hold
    the tail pass and the rowsum."""
    import concourse.bacc as bacc
    import concourse.tile as tile
    from concourse import mybir

    fp32 = mybir.dt.float32
    bf16 = mybir.dt.bfloat16
    AF = mybir.ActivationFunctionType
    OP = mybir.AluOpType

    c_go = 4 * umax
    c47 = 8 * umax
    selw = 8 * umax + 8

    nc = bacc.Bacc("TRN2", target_bir_lowering=False, debug=False)
    pa = nc.declare_dram_parameter("pa", [128, 768], bf16, isOutput=False)
    pk2 = nc.declare_dram_parameter("pk2", [21, 1920], bf16, isOutput=False)
    sel = nc.declare_dram_parameter("sel", [128, selw], bf16, isOutput=False)
    pf = nc.declare_dram_parameter("pf", [128, 4 + umax], fp32, isOutput=False)
    acc_out = nc.declare_dram_parameter(
        "acc", [128, umax + 2], fp32, isOutput=True
    )

    with tile.TileContext(nc) as tc:
        with (
            tc.tile_pool(name="io", bufs=1) as io,
            tc.tile_pool(name="work", bufs=1) as work,
            tc.tile_pool(name="psum", bufs=1, space="PSUM") as psum,
        ):
            t_pa = io.tile([128, 768], bf16)
            t_pk2 = io.tile([21, 1920], bf16)
            t_sel = io.tile([128, selw], bf16)
            t_pf = io.tile([128, 4 + umax], fp32)
            # ACT queue stays free of DMA issues so its sequencer reaches
            # the act-table load immediately
            nc.sync.dma_start(t_pa[:], pa[:])
            nc.gpsimd.dma_start(t_pk2[:], pk2[:])
            nc.sync.dma_start(t_sel[:], sel[:])
            nc.gpsimd.dma_start(t_pf[:], pf[:])

            anchors = t_pa[:, 0:APC]       # X_I^T [d, 64]
            xia = t_pa[:, 0:128]           # X_Idup^T [d, 128]
            xga = t_pa[:, 128:640]         # -2 X^T [d, 512]
            x47e = t_pa[:, 640:704]        # X_47^T [d, 64] (pads zero)
            x47 = t_pa[:, 640:768]         # X_47 dup [d, 128]

            # ---- transposed d2t (main): [g(part, 4 chunks), anchor(64)] ----
            # product+augment per chunk stay adjacent: a psum bank allows
            # only one pending accumulation group at a time
            p_d2t = psum.tile([128, 4 * APC], fp32, tag="d2t")
            for q in range(4):
                nc.tensor.matmul(
                    p_d2t[:, q * APC : (q + 1) * APC],
                    t_pa[:, 128 + q * 128 : 128 + (q + 1) * 128],
                    anchors,
                    start=True,
                    stop=False,
                )
                nc.tensor.matmul(
                    p_d2t[:, q * APC : (q + 1) * APC],
                    t_pk2[0:21, 640 + q * 128 : 640 + (q + 1) * 128],
                    t_pk2[0:21, 1152:1216],
                    start=False,
                    stop=True,
                )
            # ---- big d2 (main): [slot(128), g(512)] ----
            p_d2 = psum.tile([128, N], fp32, tag="d2")
            nc.tensor.matmul(p_d2[:], xia, xga, start=True, stop=False)
            nc.tensor.matmul(
                p_d2[:], t_pk2[0:18, 0:128], t_pk2[0:18, 128:640],
                start=False, stop=True,
            )
            # ---- tail: transposed d2t47 [g(part), 47-anchor(64)] ----
            p_d2t47 = psum.tile([128, 4 * APC], fp32, tag="d2t47")
            for q in range(4):
                nc.tensor.matmul(
                    p_d2t47[:, q * APC : (q + 1) * APC],
                    t_pa[:, 128 + q * 128 : 128 + (q + 1) * 128],
                    x47e,
                    start=True,
                    stop=False,
                )
                nc.tensor.matmul(
                    p_d2t47[:, q * APC : (q + 1) * APC],
                    t_pk2[0:4, 640 + q * 128 : 640 + (q + 1) * 128],
                    t_pk2[0:4, 1856:1920],
                    start=False,
                    stop=True,
                )
            # ---- tail: big d2 block [47-slot(128), g(512)] ----
            p_d247 = psum.tile([128, N], fp32, tag="d247")
            nc.tensor.matmul(p_d247[:], x47, xga, start=True, stop=False)
            nc.tensor.matmul(
                p_d247[:], t_pk2[0:3, 1216:1344], t_pk2[0:3, 1344:1856],
                start=False, stop=True,
            )

            # ---- sqrts (ACT) ----
            t_dpost = work.tile([128, 4 * APC], bf16, tag="dpost")
            nc.scalar.activation(t_dpost[:], p_d2t[:], AF.Sqrt)
            t_bneg = work.tile([128, N], bf16, tag="bneg")
            nc.scalar.activation(t_bneg[:], p_d2[:], AF.Sqrt, bias=t_pf[:, 0:1])
            t_dpost47 = work.tile([128, 4 * APC], bf16, tag="dpost47")
            nc.scalar.activation(t_dpost47[:], p_d2t47[:], AF.Sqrt)
            t_bneg47 = work.tile([128, N], bf16, tag="bneg47")
            nc.scalar.activation(
                t_bneg47[:], p_d247[:], AF.Sqrt, bias=t_pf[:, 1:2]
            )

            # ---- positive-distance gathers (PE) ----
            p_ab = psum.tile([128, umax], fp32, tag="ab")
            for q in range(4):
                nc.tensor.matmul(
                    p_ab[:APC, :],
                    t_dpost[:, q * APC : (q + 1) * APC],
                    t_sel[:, q * umax : (q + 1) * umax],
                    start=(q == 0),
                    stop=(q == 3),
                )
            for q in range(4):
                nc.tensor.matmul(
                    p_ab[APC:, :],
                    t_dpost[:, q * APC : (q + 1) * APC],
                    t_sel[:, c_go + q * umax : c_go + (q + 1) * umax],
                    start=(q == 0),
                    stop=(q == 3),
                )
            p_ab47 = psum.tile([128, 1], fp32, tag="ab47")
            for q in range(4):
                nc.tensor.matmul(
                    p_ab47[:APC, :],
                    t_dpost47[:, q * APC : (q + 1) * APC],
                    t_sel[:, c47 + q : c47 + q + 1],
                    start=(q == 0),
                    stop=(q == 3),
                )
            for q in range(4):
                nc.tensor.matmul(
                    p_ab47[APC:, :],
                    t_dpost47[:, q * APC : (q + 1) * APC],
                    t_sel[:, c47 + 4 + q : c47 + 5 + q],
                    start=(q == 0),
                    stop=(q == 3),
                )

            # ---- bias tiles (DVE): subtract host corrections ----
            t_ab = work.tile([128, umax], fp32, tag="ab2")
            nc.vector.tensor_tensor(
                t_ab[:], p_ab[:], t_pf[:, 2 : 2 + umax], op=OP.subtract
            )
            t_ab47 = work.tile([128, 1], fp32, tag="ab47s")
            nc.vector.tensor_tensor(
                t_ab47[:], p_ab47[:], t_pf[:, 2 + umax : 3 + umax],
                op=OP.subtract,
            )

            # ---- main loop ----
            t_acc = work.tile([128, umax + 2], fp32, tag="acc")
            t_trash_d = [
                work.tile([128, N], bf16, name=f"trd{i}", tag=f"trd{i}")
                for i in range(2)
            ]
            t_trash_a = [
                work.tile([128, N], bf16, name=f"tra{i}", tag=f"tra{i}")
                for i in range(2)
            ]
            # rowsum(d) column for the max-sum correction: a DVE max-pass
            # with bias -1e9 (max(d, -1e9) = d, so the accum is rowsum)
            nc.vector.tensor_scalar(
                t_trash_d[1][:],
                t_bneg[:],
                t_pf[:, 3 + umax : 4 + umax],
                None,
                op0=OP.max,
                op1=OP.add,
                accum_out=t_acc[:, umax + 1 : umax + 2],
            )
            nd = na = 0
            for u in range(umax):
                if u in act_us:
                    nc.scalar.activation(
                        t_trash_a[na % 2][:],
                        t_bneg[:],
                        AF.Relu,
                        bias=t_ab[:, u : u + 1],
                        scale=-1.0,
                        accum_out=t_acc[:, u : u + 1],
                    )
                    na += 1
                else:
                    nc.vector.tensor_scalar(
                        t_trash_d[nd % 2][:],
                        t_bneg[:],
                        t_ab[:, u : u + 1],
                        None,
                        op0=OP.max,
                        op1=OP.add,
                        accum_out=t_acc[:, u : u + 1],
                    )
                    nd += 1
            # tail pass: big-class ranks for this core's rank pair (ACT)
            nc.scalar.activation(
                t_trash_a[na % 2][:],
                t_bneg47[:],
                AF.Relu,
                bias=t_ab47[:, 0:1],
                scale=-1.0,
                accum_out=t_acc[:, umax : umax + 1],
            )

            nc.sync.dma_start(acc_out[:], t_acc[:])

    nc.finalize()
    return nc


def plan(outputs, labels, margin, n_act=2):
    """Build (nc, in_maps, umax_m, count); shared by kernel() and test."""
    X64 = np.asarray(outputs, dtype=np.float64)
    lab = np.asarray(labels).astype(np.int64).reshape(-1)
    margin = float(margin)
    assert X64.shape == (N, D) and lab.shape == (N,)

    Xb = X64.astype(BF)                      # round once
    Xw = Xb.astype(np.float64)               # exact value of the rounding
    r = (Xw * Xw).sum(1)                     # row norms of the rounded X

    nclass = max(C, int(lab.max()) + 1)
    m = np.bincount(lab, minlength=nclass)
    jmax = int(m.max())
    cbig = int(m.argmax())
    m2 = int(np.sort(m)[-2])
    umax = (m2 + 1) // 2                     # main ordinal pairs
    tp = max(0, (jmax - 2 * umax + 1) // 2)  # tail rank pairs
    assert tp <= NCORES, (jmax, m2, tp)
    I47 = np.flatnonzero(lab == cbig)
    n47 = len(I47)
    assert n47 <= APC
    count = float(sum(int(mc) * (int(mc) - 1) * (N - int(mc)) for mc in m))

    rank = np.zeros(N, dtype=np.int64)
    cnt = {}
    for j in range(N):
        c = int(lab[j])
        rank[j] = cnt.get(c, 0)
        cnt[c] = cnt.get(c, 0) + 1
    G = np.zeros((N, 2 * umax), dtype=np.float64)
    ok = rank < 2 * umax
    G[np.arange(N)[ok], rank[ok]] = 1.0
    GE, GO = G[:, 0::2], G[:, 1::2]  # [512, umax] each

    n_act = max(0, min(n_act, umax))
    act_us = frozenset(range(umax - n_act, umax))
    global _LAST_ACT_US
    _LAST_ACT_US = act_us

    key = (umax, margin, act_us)
    if key not in _CACHE:
        _CACHE[key] = _build_program(umax, margin, act_us)
    nc = _CACHE[key]

    def chunked(A, cols):
        # [512, cols] -> [128, 4*cols] with chunk q at cols [q*cols:(q+1)*cols]
        return A.reshape(4, 128, cols).transpose(1, 0, 2).reshape(128, 4 * cols)

    r_hi = r.astype(BF)
    r_lo = (r - r_hi.astype(np.float64)).astype(BF)
    v = r + FUDGE
    v_hi = v.astype(BF)
    v_lo = (v - v_hi.astype(np.float64)).astype(BF)

    onehot = lab[None, :] == np.arange(nclass)[:C, None]  # [16, 512]
    c_go = 4 * umax
    c47 = 8 * umax
    selw = 8 * umax + 8

    # X47 dup block [d, 128]: cols 0:n47 even copy, 64:64+n47 odd copy
    X47blk = np.zeros((D, 128), dtype=BF)
    X47blk[:, 0:n47] = Xb[I47].T
    X47blk[:, 64 : 64 + n47] = Xb[I47].T
    pad47 = np.ones(128, dtype=bool)
    pad47[0:n47] = False
    pad47[64 : 64 + n47] = False
    r47 = np.zeros(128)
    r47[0:n47] = r[I47]
    r47[64 : 64 + n47] = r[I47]
    v47 = np.zeros(64)
    v47[0:n47] = v[I47]
    v47_hi = v47.astype(BF)
    v47_lo = (v47 - v47_hi.astype(np.float64)).astype(BF)

    # nclasses_gt[rank] = #classes with m > rank
    nclasses_gt = np.zeros(N + 2, dtype=np.int64)
    for mc in m:
        nclasses_gt[: int(mc)] += 1

    in_maps = []
    for c in range(NCORES):
        I = np.arange(c * APC, (c + 1) * APC)
        Idup = np.concatenate([I, I])

        PA = np.empty((128, 768), dtype=BF)
        PA[:, 0:128] = Xb[Idup].T
        PA[:, 128:640] = (-2.0 * Xw).astype(BF).T  # exact: power-of-2 scale
        PA[:, 640:768] = X47blk

        PK2 = np.zeros((21, 1920), dtype=BF)
        # main big augment: lhsT [18,128] at 0:128, rhs [18,512] at 128:640
        PK2[0, 0:128] = 1.0
        PK2[1, 0:128] = 1.0
        PK2[2:18, 0:128] = np.where(onehot[:, Idup], MASK, 0.0)
        PK2[0, 128:640] = r_hi
        PK2[1, 128:640] = r_lo
        PK2[2:18, 128:640] = onehot.astype(np.float64)
        # main transposed augment: lhsT [21,128] per chunk at 640+q*128,
        # rhs [21,64] at 1152:1216; rows 4..20 put MASK on CROSS-class
        for q in range(4):
            s = 640 + q * 128
            PK2[0, s : s + 128] = r_hi[q * 128 : (q + 1) * 128]
            PK2[1, s : s + 128] = r_lo[q * 128 : (q + 1) * 128]
            PK2[2, s : s + 128] = 1.0
            PK2[3, s : s + 128] = 1.0
            PK2[4, s : s + 128] = MASK
            PK2[5:21, s : s + 128] = np.where(
                onehot[:, q * 128 : (q + 1) * 128], -MASK, 0.0
            )
        PK2[0, 1152:1216] = 1.0
        PK2[1, 1152:1216] = 1.0
        PK2[2, 1152:1216] = v_hi[I]
        PK2[3, 1152:1216] = v_lo[I]
        PK2[4, 1152:1216] = 1.0
        PK2[5:21, 1152:1216] = onehot[:, I].astype(np.float64)
        # tail big augment: lhsT [3,128] at 1216:1344, rhs [3,512] at 1344:1856
        PK2[0, 1216:1344] = 1.0
        PK2[1, 1216:1344] = 1.0
        PK2[2, 1216:1344] = MASK
        PK2[0, 1344:1856] = r_hi
        PK2[1, 1344:1856] = r_lo
        PK2[2, 1344:1856] = onehot[cbig].astype(np.float64)
        # tail transposed augment rhs [4,64] at 1856:1920 (lhsT reuses the
        # main chunk rows 0:4)
        PK2[0, 1856:1920] = 1.0
        PK2[1, 1856:1920] = 1.0
        PK2[2, 1856:1920] = v47_hi
        PK2[3, 1856:1920] = v47_lo

        SEL = np.zeros((128, selw), dtype=BF)
        SEL[:, 0 : 4 * umax] = chunked(GE, umax)
        SEL[:, c_go : c_go + 4 * umax] = chunked(GO, umax)
        # tail selectors: core c handles ranks (2*umax+2c, 2*umax+2c+1)
        if c < tp:
            re_, ro_ = 2 * umax + 2 * c, 2 * umax + 2 * c + 1
            S47e = ((lab == cbig) & (rank == re_)).astype(np.float64)
            S47o = ((lab == cbig) & (rank == ro_)).astype(np.float64)
            SEL[:, c47 : c47 + 4] = S47e.reshape(4, 128).T
            SEL[:, c47 + 4 : c47 + 8] = S47o.reshape(4, 128).T

        PF = np.zeros((128, 4 + umax), dtype=np.float32)
        PF[:, 3 + umax] = -PADBIAS
        PF[:, 0] = r[Idup]
        PF[:, 1] = r47
        rho = np.empty((128, umax), dtype=np.int64)
        uu = np.arange(umax)
        rho[:64] = 2 * uu[None, :]
        rho[64:] = 2 * uu[None, :] + 1
        own = m[lab[Idup]]
        cgar = nclasses_gt[rho] - (rho < own[:, None])
        PF[:, 2 : 2 + umax] = (MVAL * cgar - margin).astype(np.float32)
        PF[:, 2 + umax] = np.where(pad47, PADBIAS, -margin)

        in_maps.append({"pa": PA, "pk2": PK2, "sel": SEL, "pf": PF})

    return nc, in_maps, umax, count


_LAST_ACT_US = frozenset()


def reduce_results(results, umax, count):
    # ACT cols (act_us and the tail col umax) hold relu sums directly; DVE
    # cols hold sum_g max(d_g, a_u) and need the rowsum col (umax+1)
    # subtracted.
    total = 0.0
    for c in range(NCORES):
        acc = results[c]["acc"].astype(np.float64)  # [128, umax+2]
        rs = acc[:, umax + 1].sum()
        for u in range(umax):
            cs = acc[:, u].sum()
            total += cs if u in _LAST_ACT_US else cs - rs
        total += acc[:, umax].sum()  # tail col
    return np.float32(total / count)


def kernel(outputs, labels, margin):
    from concourse.bass_utils import run_bass_kernel_spmd

    nc, in_maps, umax, count = plan(outputs, labels, margin)
    res = run_bass_kernel_spmd(nc, in_maps, list(range(NCORES)))
    loss = reduce_results(res.results, umax, count)
    return (loss, 0.0, 0.0, 0.0)
